# revision 4
# baseline (speedup 1.0000x reference)
"""DOMINO loss (DiceCE + penalty) Trainium2 kernel v5, 8-core data-parallel.

Per-core math (H-sharded): with one-hot T1h (host-built, fp8) and softmax
p = e/s computed on device:
    Gp[n] = sum_px t1h (x) p        (12x12 Gram via TensorE, PSUM-accumulated)
    inter = diag(Gp), pred_o = col-sums(Gp), penalty = BETA/npix <M, sum_n Gp>
    CE    = mean(log s) - mean(x_t)   [log-sum on device, x_t-sum on host fp32]
    ground_o = host bincount.

Engine split (per 32 tiles/core, cost-model validated):
    ScalarE: exp fp8->bf16 (1465ns/tile) + per-4-tile-group Ln(s)+accum and
             r = exp(-L) (~350ns/tile)                    ~58us
    DVE:     4-op add-tree for s (974ns) + p = e*r bcast mul at 2x (860ns)
                                                          ~59us
    PE:      one-hot Gram, 16 windows x N=96/tile         ~22us
    DMA:     x fp8 + one-hot fp8, contiguous 1536B descriptors   ~35us

vs baseline: mul ran at 1x (fp8 out), x-Gram doubled PE+rhs, xin DMA paid the
<512B descriptor penalty, exp+Ln+tree left DVE at 95us busy.
"""

import numpy as np
import ml_dtypes

import concourse.bacc as bacc
import concourse.mybir as mybir
import concourse.tile as tile
from concourse.bass_utils import run_bass_kernel_spmd

FP8 = ml_dtypes.float8_e4m3

NCORES = 8
N, C, H, W, Z = 2, 12, 128, 128, 128
SMOOTH = 1e-5
BETA = 3.0

HSH = H // NCORES          # 16 H-rows per core
PXN = HSH * W * Z          # pixels per (core, n) = 262144
COLS = PXN // 128          # px-cols per n = 2048
WT = 128                   # px-cols per tile
TPN = COLS // WT           # tiles per n = 16
NT = N * TPN               # tiles per core = 32
NPIX = N * H * W * Z       # total pixels
MMW = 8                    # px-cols per matmul window (M = 12*8 = 96)
NW = WT // MMW             # windows per tile = 16
GRP = 4                    # tiles per Ln/negexp batch group
NGRP = NT // GRP           # 8 groups

# Mitchell log2 bit-trick: for s > 0 in bf16, bits = E*128 + M and
# ln s ~= bits*(ln2/128) - 127*ln2 + c, with c the mean sawtooth correction
# E[ln m - (m-1)ln2] over mantissas (0.039721 for log-uniform mantissas).
MITCH_A = 0.0054152123481245725
MITCH_B = -87.98997116027313

_CACHE = {}
_ABLATE = set()            # dev-only: {"pe", "dve", "act"} to skip engine work


def _build_nc(reps=1):
    nc = bacc.Bacc(None, target_bir_lowering=False)
    dt = mybir.dt
    AF = mybir.ActivationFunctionType
    xin = nc.declare_dram_parameter("xin", [NT, 128, 12 * WT], dt.float8e4, isOutput=False)
    tin = nc.declare_dram_parameter("tin", [NT, 128, 12 * WT], dt.float8e4, isOutput=False)
    gout = nc.declare_dram_parameter("gout", [96, 192], dt.float32, isOutput=True)
    lout = nc.declare_dram_parameter("lout", [128, NGRP], dt.float32, isOutput=True)

    with tile.TileContext(nc) as tc:
        with (
            tc.tile_pool(name="px", bufs=6) as pxpool,
            tc.tile_pool(name="th", bufs=10) as thpool,
            tc.tile_pool(name="e", bufs=10) as epool,
            tc.tile_pool(name="p", bufs=6) as ppool,
            tc.tile_pool(name="tree", bufs=4) as treepool,
            tc.tile_pool(name="grp", bufs=3) as grppool,
            tc.tile_pool(name="persist", bufs=1) as perspool,
            tc.tile_pool(name="psum", bufs=1, space="PSUM") as psumpool,
        ):
            lacc = perspool.tile([128, NGRP], dt.float32)
            negb = perspool.tile([128, 1], dt.float32)
            nc.vector.memset(negb[:], -MITCH_B)
            g_ps = [
                psumpool.tile([96, 96], dt.float32, tag=f"g{n}", name=f"g{n}")
                for n in range(N)
            ]

            from contextlib import nullcontext

            loop_ctx = tc.For_i(0, reps, 1) if reps > 1 else nullcontext()
            with loop_ctx:
                for g in range(NGRP):
                    s4 = grppool.tile([128, GRP, NW, MMW], dt.bfloat16, tag="s4")
                    r4 = grppool.tile([128, GRP, NW, MMW], dt.bfloat16, tag="r4")
                    es, ths = [], []
                    for gi in range(GRP):
                        t = g * GRP + gi
                        xt = pxpool.tile([128, NW, 12, MMW], dt.float8e4, tag="px")
                        nc.sync.dma_start(
                            xt[:], xin[t].rearrange("p (a c w) -> p a c w", a=NW, c=12)
                        )
                        th = thpool.tile([128, NW, 12, MMW], dt.float8e4, tag="th")
                        nc.sync.dma_start(
                            th[:], tin[t].rearrange("p (a c w) -> p a c w", a=NW, c=12)
                        )
                        ths.append(th)

                        if "act" not in _ABLATE:
                            e = epool.tile([128, NW, 12, MMW], dt.bfloat16, tag="e")
                            nc.scalar.activation(e[:], xt[:], AF.Exp)
                            es.append(e)

                        if "dve" not in _ABLATE:
                            t6 = treepool.tile([128, NW, 6, MMW], dt.bfloat16, tag="t6")
                            nc.vector.tensor_add(t6[:], e[:, :, 0:6, :], e[:, :, 6:12, :])
                            t3 = treepool.tile([128, NW, 3, MMW], dt.bfloat16, tag="t3")
                            nc.vector.tensor_add(t3[:], t6[:, :, 0:3, :], t6[:, :, 3:6, :])
                            st = s4[:, gi]
                            nc.gpsimd.tensor_add(st, t3[:, :, 0, :], t3[:, :, 1, :])
                            nc.gpsimd.tensor_add(st, st, t3[:, :, 2, :])

                    if "act" not in _ABLATE:
                        # batched over the 4-tile group: L = ln(s) via the
                        # Mitchell bit-trick on DVE (keeps ScalarE exp-only ->
                        # one activation table, no per-group LUT reloads);
                        # accum_out gives the CE log-sum. r = 1/s = exp(-L)
                        # on ScalarE.
                        # L' = bits * a  (true ln s = L' + MITCH_B); the +B is
                        # folded into the exp bias below and into the host-side
                        # log-sum assembly (accum semantics: s2 + sum(out)).
                        L4 = grppool.tile([128, GRP * NW * MMW], dt.float32, tag="L4")
                        sbits = s4[:].rearrange("p a b c -> p (a b c)").bitcast(dt.uint16)
                        nc.vector.tensor_scalar(
                            L4[:], sbits, MITCH_A, 0.0,
                            mybir.AluOpType.mult, mybir.AluOpType.add,
                            accum_out=lacc[:, g : g + 1],
                        )
                        nc.scalar.activation(
                            r4[:].rearrange("p a b c -> p (a b c)"), L4[:],
                            AF.Exp, scale=-1.0, bias=negb[:],
                        )

                    for gi in range(GRP):
                        t = g * GRP + gi
                        n = t // TPN
                        if "dve" not in _ABLATE:
                            p = ppool.tile([128, NW, 12, MMW], dt.bfloat16, tag="p")
                            nc.vector.tensor_mul(
                                p[:], es[gi][:],
                                r4[:, gi].unsqueeze(2).broadcast_to([128, NW, 12, MMW]),
                            )

                        if "pe" not in _ABLATE:
                            for w8 in range(NW):
                                first = (t % TPN == 0) and w8 == 0
                                last = (t % TPN == TPN - 1) and w8 == NW - 1
                                nc.tensor.matmul(
                                    g_ps[n][:, :], ths[gi][:, w8], p[:, w8],
                                    start=first, stop=last,
                                )

            gsb = perspool.tile([96, 192], dt.float32)
            if "pe" not in _ABLATE:
                for n in range(N):
                    nc.vector.tensor_copy(gsb[:, 96 * n : 96 * (n + 1)], g_ps[n][:])
            else:
                nc.vector.memset(gsb[:], 0.0)
            nc.sync.dma_start(gout[:], gsb[:])
            nc.sync.dma_start(lout[:], lacc[:])

    nc.finalize()
    return nc


def _prep_core(x, t, k):
    """Per-core device arrays. x: (N,C,H,W,Z) f32, t: (N,H,W,Z) int."""
    xc = np.ascontiguousarray(x[:, :, HSH * k : HSH * (k + 1)])      # (2,12,16,128,128)
    xd = (
        xc.reshape(N, C, 128, TPN, NW, MMW)
        .transpose(0, 3, 2, 4, 1, 5)                                  # n,t16,p,w8,c,w
        .reshape(NT, 128, 12 * WT)
        .astype(FP8)
    )
    tc_ = (
        t[:, HSH * k : HSH * (k + 1)]
        .reshape(N, 128, TPN, NW, MMW)
        .transpose(0, 2, 1, 3, 4)                                     # n,t16,p,w8,w
        .reshape(NT, 128, NW, MMW)
    )
    th = tc_[:, :, :, None, :] == np.arange(C, dtype=tc_.dtype)[None, None, None, :, None]
    thd = th.astype(FP8).reshape(NT, 128, 12 * WT)
    return xd, thd


def _decode(results):
    """Sum per-core G blocks -> Gp[n] (12x12) + logsum."""
    Gp = np.zeros((N, C, C), np.float64)
    logsum = 0.0
    for res in results:
        g = res["gout"].astype(np.float64)                            # [96, 192]
        for n in range(N):
            blk = g[:, 96 * n : 96 * (n + 1)].reshape(C, MMW, C, MMW)
            Gp[n] += np.einsum("awbw->ab", blk)
        logsum += float(res["lout"].astype(np.float64).sum())
    logsum += MITCH_B * NPIX    # accum carries only the bits*a part of ln s
    return Gp, logsum


def run(inputs, trace=False):
    x = np.asarray(inputs["input"], dtype=np.float32)
    t = np.asarray(inputs["target"])
    Mp = np.asarray(inputs["matrix_penalty"], dtype=np.float32)
    tt = np.asarray(t[:, 0])                                          # (N,H,W,Z) int

    if "nc" not in _CACHE:
        _CACHE["nc"] = _build_nc()
    nc = _CACHE["nc"]

    in_maps = []
    for k in range(NCORES):
        xd, thd = _prep_core(x, tt, k)
        in_maps.append({"xin": xd, "tin": thd})

    res = run_bass_kernel_spmd(nc, in_maps, core_ids=list(range(NCORES)), trace=trace)
    Gp, logsum = _decode(res.results)

    ground_o = np.stack(
        [np.bincount(tt[n].ravel().astype(np.int64), minlength=C) for n in range(N)]
    ).astype(np.float64)
    inter = np.einsum("ncc->nc", Gp)
    pred_o = Gp.sum(axis=1)
    # CE x-term on host from the full-precision input
    xt_sum = float(
        np.take_along_axis(x, tt[:, None].astype(np.int64), axis=1).sum(dtype=np.float64)
    )

    ce = (logsum - xt_sum) / NPIX
    dice = np.mean(1.0 - (2.0 * inter + SMOOTH) / (ground_o + pred_o + SMOOTH))
    pen = BETA / NPIX * float((Mp[None] * Gp).sum())
    loss = np.float32(ce + dice + pen)
    return loss, res


def kernel(**inputs):
    return run(inputs)[0]


# revision 5
# speedup vs baseline: 1.2330x; 1.2330x over previous
"""DOMINO loss (DiceCE + penalty) Trainium2 kernel v9, 8-core data-parallel.

v7 + group-batched DVE: the add-tree and the normalize-mul run once per
4-tile group on [128, 4, NW, C, MMW] super-tiles, amortizing per-op init
(~60 cyc) and SEQ issue overhead 4x.  DVE ~50us, ACT ~53us, DMA ~39us,
PE ~30us (cost model).
"""

import numpy as np
import ml_dtypes

import concourse.bacc as bacc
import concourse.mybir as mybir
import concourse.tile as tile
from concourse.bass_utils import run_bass_kernel_spmd

FP8 = ml_dtypes.float8_e4m3

NCORES = 8
N, C, H, W, Z = 2, 12, 128, 128, 128
SMOOTH = 1e-5
BETA = 3.0

HSH = H // NCORES          # 16 H-rows per core
PXN = HSH * W * Z          # pixels per (core, n) = 262144
COLS = PXN // 128          # px-cols per n = 2048
WT = 128                   # px-cols per tile
TPN = COLS // WT           # tiles per n = 16
NT = N * TPN               # tiles per core = 32
NPIX = N * H * W * Z       # total pixels
MMW = 8                    # px-cols per matmul window (M = 12*8 = 96)
NW = WT // MMW             # windows per tile = 16
GRP = 4                    # tiles per batch group
NGRP = NT // GRP           # 8 groups

# Mitchell log2 bit-trick: for s > 0 in bf16, bits = E*128 + M and
# ln s ~= bits*(ln2/128) - 127*ln2 + c, c = mean sawtooth correction.
MITCH_A = 0.0054152123481245725
MITCH_B = -87.98997116027313

_CACHE = {}
_ABLATE = set()


def _build_nc(reps=1):
    nc = bacc.Bacc(None, target_bir_lowering=False)
    dt = mybir.dt
    AF = mybir.ActivationFunctionType
    xin = nc.declare_dram_parameter("xin", [NT, 128, 12 * WT], dt.float8e4, isOutput=False)
    tin = nc.declare_dram_parameter("tin", [NT, 128, 12 * WT], dt.float8e4, isOutput=False)
    gout = nc.declare_dram_parameter("gout", [96, 192], dt.float32, isOutput=True)
    lout = nc.declare_dram_parameter("lout", [128, NGRP], dt.float32, isOutput=True)

    SG = [128, GRP, NW, 12, MMW]      # group super-tile shape

    with tile.TileContext(nc) as tc:
        with (
            tc.tile_pool(name="px", bufs=6) as pxpool,
            tc.tile_pool(name="th", bufs=10) as thpool,
            tc.tile_pool(name="e", bufs=3) as epool,
            tc.tile_pool(name="p", bufs=2) as ppool,
            tc.tile_pool(name="tree", bufs=2) as treepool,
            tc.tile_pool(name="grp", bufs=3) as grppool,
            tc.tile_pool(name="persist", bufs=1) as perspool,
            tc.tile_pool(name="psum", bufs=1, space="PSUM") as psumpool,
        ):
            lacc = perspool.tile([128, NGRP], dt.float32)
            negb = perspool.tile([128, 1], dt.float32)
            nc.vector.memset(negb[:], -MITCH_B)
            g_ps = [
                psumpool.tile([96, 96], dt.float32, tag=f"g{n}", name=f"g{n}")
                for n in range(N)
            ]

            from contextlib import nullcontext

            loop_ctx = tc.For_i(0, reps, 1) if reps > 1 else nullcontext()
            with loop_ctx:
                for g in range(NGRP):
                    s4 = grppool.tile([128, GRP, NW, MMW], dt.bfloat16, tag="s4")
                    r4 = grppool.tile([128, GRP, NW, MMW], dt.bfloat16, tag="r4")
                    eg = epool.tile(SG, dt.bfloat16, tag="e")
                    ths = []
                    for gi in range(GRP):
                        t = g * GRP + gi
                        xt = pxpool.tile([128, NW, 12, MMW], dt.float8e4, tag="px")
                        nc.sync.dma_start(
                            xt[:], xin[t].rearrange("p (a c w) -> p a c w", a=NW, c=12)
                        )
                        th = thpool.tile([128, NW, 12, MMW], dt.float8e4, tag="th")
                        nc.sync.dma_start(
                            th[:], tin[t].rearrange("p (a c w) -> p a c w", a=NW, c=12)
                        )
                        ths.append(th)
                        if "act" not in _ABLATE:
                            nc.scalar.activation(eg[:, gi], xt[:], AF.Exp)

                    if "dve" not in _ABLATE:
                        for h0 in range(0, GRP, 2):
                            h1 = h0 + 2
                            t6 = treepool.tile([128, 2, NW, 6, MMW], dt.bfloat16, tag="t6")
                            nc.vector.tensor_add(
                                t6[:], eg[:, h0:h1, :, 0:6, :], eg[:, h0:h1, :, 6:12, :]
                            )
                            t3 = treepool.tile([128, 2, NW, 3, MMW], dt.bfloat16, tag="t3")
                            nc.vector.tensor_add(
                                t3[:], t6[:, :, :, 0:3, :], t6[:, :, :, 3:6, :]
                            )
                            sv = s4[:, h0:h1]
                            nc.vector.tensor_add(sv, t3[:, :, :, 0, :], t3[:, :, :, 1, :])
                            nc.vector.tensor_add(sv, sv, t3[:, :, :, 2, :])

                    if "act" not in _ABLATE:
                        # CE log-sum via Mitchell bits (DVE, accum); r = 1/s
                        # directly from the bits on ScalarE.
                        L4 = grppool.tile([128, GRP * NW * MMW], dt.bfloat16, tag="L4")
                        sbits = s4[:].rearrange("p a b c -> p (a b c)").bitcast(dt.uint16)
                        nc.vector.tensor_scalar(
                            L4[:], sbits, MITCH_A, 0.0,
                            mybir.AluOpType.mult, mybir.AluOpType.add,
                            accum_out=lacc[:, g : g + 1],
                        )
                        nc.scalar.activation(
                            r4[:].rearrange("p a b c -> p (a b c)"), sbits,
                            AF.Exp, scale=-MITCH_A, bias=negb[:],
                        )

                    if "dve" not in _ABLATE:
                        pg = ppool.tile(SG, dt.bfloat16, tag="p")
                        nc.vector.tensor_mul(
                            pg[:], eg[:], r4[:].unsqueeze(3).broadcast_to(SG)
                        )

                    if "pe" not in _ABLATE:
                        for gi in range(GRP):
                            t = g * GRP + gi
                            n = t // TPN
                            for w8 in range(NW):
                                first = (t % TPN == 0) and w8 == 0
                                last = (t % TPN == TPN - 1) and w8 == NW - 1
                                nc.tensor.matmul(
                                    g_ps[n][:, :], ths[gi][:, w8], pg[:, gi, w8],
                                    start=first, stop=last,
                                )

            gsb = perspool.tile([96, 192], dt.float32)
            if "pe" not in _ABLATE:
                for n in range(N):
                    nc.vector.tensor_copy(gsb[:, 96 * n : 96 * (n + 1)], g_ps[n][:])
            else:
                nc.vector.memset(gsb[:], 0.0)
            nc.sync.dma_start(gout[:], gsb[:])
            nc.sync.dma_start(lout[:], lacc[:])

    nc.finalize()
    return nc


def _prep_core(x, t, k):
    """Per-core device arrays. x: (N,C,H,W,Z) f32, t: (N,H,W,Z) int."""
    xc = np.ascontiguousarray(x[:, :, HSH * k : HSH * (k + 1)])      # (2,12,16,128,128)
    xd = (
        xc.reshape(N, C, 128, TPN, NW, MMW)
        .transpose(0, 3, 2, 4, 1, 5)                                  # n,t16,p,w8,c,w
        .reshape(NT, 128, 12 * WT)
        .astype(FP8)
    )
    tc_ = (
        t[:, HSH * k : HSH * (k + 1)]
        .reshape(N, 128, TPN, NW, MMW)
        .transpose(0, 2, 1, 3, 4)                                     # n,t16,p,w8,w
        .reshape(NT, 128, NW, MMW)
    )
    th = tc_[:, :, :, None, :] == np.arange(C, dtype=tc_.dtype)[None, None, None, :, None]
    thd = th.astype(FP8).reshape(NT, 128, 12 * WT)
    return xd, thd


def _decode(results):
    """Sum per-core G blocks -> Gp[n] (12x12) + logsum."""
    Gp = np.zeros((N, C, C), np.float64)
    logsum = 0.0
    for res in results:
        g = res["gout"].astype(np.float64)                            # [96, 192]
        for n in range(N):
            blk = g[:, 96 * n : 96 * (n + 1)].reshape(C, MMW, C, MMW)
            Gp[n] += np.einsum("awbw->ab", blk)
        logsum += float(res["lout"].astype(np.float64).sum())
    logsum += MITCH_B * NPIX    # accum carries only the bits*a part of ln s
    return Gp, logsum


def run(inputs, trace=False):
    x = np.asarray(inputs["input"], dtype=np.float32)
    t = np.asarray(inputs["target"])
    Mp = np.asarray(inputs["matrix_penalty"], dtype=np.float32)
    tt = np.asarray(t[:, 0])                                          # (N,H,W,Z) int

    if "nc" not in _CACHE:
        _CACHE["nc"] = _build_nc()
    nc = _CACHE["nc"]

    in_maps = []
    for k in range(NCORES):
        xd, thd = _prep_core(x, tt, k)
        in_maps.append({"xin": xd, "tin": thd})

    res = run_bass_kernel_spmd(nc, in_maps, core_ids=list(range(NCORES)), trace=trace)
    Gp, logsum = _decode(res.results)

    ground_o = np.stack(
        [np.bincount(tt[n].ravel().astype(np.int64), minlength=C) for n in range(N)]
    ).astype(np.float64)
    inter = np.einsum("ncc->nc", Gp)
    pred_o = Gp.sum(axis=1)
    xt_sum = float(
        np.take_along_axis(x, tt[:, None].astype(np.int64), axis=1).sum(dtype=np.float64)
    )

    ce = (logsum - xt_sum) / NPIX
    dice = np.mean(1.0 - (2.0 * inter + SMOOTH) / (ground_o + pred_o + SMOOTH))
    pen = BETA / NPIX * float((Mp[None] * Gp).sum())
    loss = np.float32(ce + dice + pen)
    return loss, res


def kernel(**inputs):
    return run(inputs)[0]


# revision 6
# speedup vs baseline: 1.2567x; 1.0192x over previous
"""DOMINO loss (DiceCE + penalty) Trainium2 kernel v10, 8-core data-parallel.

v7 + group-batched DVE: the add-tree and the normalize-mul run once per
4-tile group on [128, 4, NW, C, MMW] super-tiles, amortizing per-op init
(~60 cyc) and SEQ issue overhead 4x.  DVE ~50us, ACT ~53us, DMA ~39us,
PE ~30us (cost model).
"""

import numpy as np
import ml_dtypes

import concourse.bacc as bacc
import concourse.mybir as mybir
import concourse.tile as tile
from concourse.bass_utils import run_bass_kernel_spmd

FP8 = ml_dtypes.float8_e4m3

NCORES = 8
N, C, H, W, Z = 2, 12, 128, 128, 128
SMOOTH = 1e-5
BETA = 3.0

HSH = H // NCORES          # 16 H-rows per core
PXN = HSH * W * Z          # pixels per (core, n) = 262144
COLS = PXN // 128          # px-cols per n = 2048
WT = 128                   # px-cols per tile
TPN = COLS // WT           # tiles per n = 16
NT = N * TPN               # tiles per core = 32
NPIX = N * H * W * Z       # total pixels
MMW = 8                    # px-cols per matmul window (M = 12*8 = 96)
NW = WT // MMW             # windows per tile = 16
GRP = 4                    # tiles per batch group
NGRP = NT // GRP           # 8 groups

# Mitchell log2 bit-trick: for s > 0 in bf16, bits = E*128 + M and
# ln s ~= bits*(ln2/128) - 127*ln2 + c, c = mean sawtooth correction.
MITCH_A = 0.0054152123481245725
MITCH_B = -87.98997116027313

_CACHE = {}
_ABLATE = set()


def _build_nc(reps=1):
    nc = bacc.Bacc(None, target_bir_lowering=False)
    dt = mybir.dt
    AF = mybir.ActivationFunctionType
    xin = nc.declare_dram_parameter("xin", [NT, 128, 12 * WT], dt.float8e4, isOutput=False)
    tin = nc.declare_dram_parameter("tin", [NT, 128, 12 * WT], dt.float8e4, isOutput=False)
    gout = nc.declare_dram_parameter("gout", [96, 192], dt.float32, isOutput=True)
    lout = nc.declare_dram_parameter("lout", [128, NGRP], dt.float32, isOutput=True)

    SG = [128, GRP, NW, 12, MMW]      # group super-tile shape

    with tile.TileContext(nc) as tc:
        with (
            tc.tile_pool(name="px", bufs=6) as pxpool,
            tc.tile_pool(name="th", bufs=10) as thpool,
            tc.tile_pool(name="e", bufs=3) as epool,
            tc.tile_pool(name="p", bufs=2) as ppool,
            tc.tile_pool(name="tree", bufs=2) as treepool,
            tc.tile_pool(name="grp", bufs=3) as grppool,
            tc.tile_pool(name="persist", bufs=1) as perspool,
            tc.tile_pool(name="psum", bufs=1, space="PSUM") as psumpool,
        ):
            lacc = perspool.tile([128, NGRP], dt.float32)
            negb = perspool.tile([128, 1], dt.float32)
            nc.vector.memset(negb[:], -MITCH_B)
            g_ps = [
                psumpool.tile([96, 96], dt.float32, tag=f"g{n}", name=f"g{n}")
                for n in range(N)
            ]

            from contextlib import nullcontext

            loop_ctx = tc.For_i(0, reps, 1) if reps > 1 else nullcontext()
            with loop_ctx:
                for g in range(NGRP):
                    s4 = grppool.tile([128, GRP, NW, MMW], dt.bfloat16, tag="s4")
                    r4 = grppool.tile([128, GRP, NW, MMW], dt.bfloat16, tag="r4")
                    eg = epool.tile(SG, dt.bfloat16, tag="e")
                    ths = []
                    for gi in range(GRP):
                        t = g * GRP + gi
                        xt = pxpool.tile([128, NW, 12, MMW], dt.float8e4, tag="px")
                        nc.sync.dma_start(
                            xt[:], xin[t].rearrange("p (a c w) -> p a c w", a=NW, c=12)
                        )
                        th = thpool.tile([128, NW, 12, MMW], dt.float8e4, tag="th")
                        nc.sync.dma_start(
                            th[:], tin[t].rearrange("p (a c w) -> p a c w", a=NW, c=12)
                        )
                        ths.append(th)
                        if "act" not in _ABLATE:
                            nc.scalar.activation(eg[:, gi], xt[:], AF.Exp)

                    if "dve" not in _ABLATE:
                        for h0 in range(0, GRP, 2):
                            h1 = h0 + 2
                            t6 = treepool.tile([128, 2, NW, 6, MMW], dt.bfloat16, tag="t6")
                            nc.vector.tensor_add(
                                t6[:], eg[:, h0:h1, :, 0:6, :], eg[:, h0:h1, :, 6:12, :]
                            )
                            t3 = treepool.tile([128, 2, NW, 3, MMW], dt.bfloat16, tag="t3")
                            nc.vector.tensor_add(
                                t3[:], t6[:, :, :, 0:3, :], t6[:, :, :, 3:6, :]
                            )
                            sv = s4[:, h0:h1]
                            nc.vector.tensor_add(sv, t3[:, :, :, 0, :], t3[:, :, :, 1, :])
                            nc.vector.tensor_add(sv, sv, t3[:, :, :, 2, :])

                    if "act" not in _ABLATE:
                        # CE log-sum via Mitchell bits (DVE, accum); r = 1/s
                        # directly from the bits on ScalarE.
                        L4 = grppool.tile([128, GRP * NW * MMW], dt.bfloat16, tag="L4")
                        sbits = s4[:].rearrange("p a b c -> p (a b c)").bitcast(dt.uint16)
                        nc.vector.tensor_scalar(
                            L4[:], sbits, MITCH_A, 0.0,
                            mybir.AluOpType.mult, mybir.AluOpType.add,
                            accum_out=lacc[:, g : g + 1],
                        )
                        nc.scalar.activation(
                            r4[:].rearrange("p a b c -> p (a b c)"), sbits,
                            AF.Exp, scale=-MITCH_A, bias=negb[:],
                        )

                    if "dve" not in _ABLATE:
                        SH = [128, 2, NW, 12, MMW]
                        pg = ppool.tile(SG, dt.bfloat16, tag="p")
                        for h0 in range(0, GRP, 2):
                            h1 = h0 + 2
                            nc.vector.tensor_mul(
                                pg[:, h0:h1], eg[:, h0:h1],
                                r4[:, h0:h1].unsqueeze(3).broadcast_to(SH),
                            )

                    if "pe" not in _ABLATE:
                        for gi in range(GRP):
                            t = g * GRP + gi
                            n = t // TPN
                            for w8 in range(NW):
                                first = (t % TPN == 0) and w8 == 0
                                last = (t % TPN == TPN - 1) and w8 == NW - 1
                                nc.tensor.matmul(
                                    g_ps[n][:, :], ths[gi][:, w8], pg[:, gi, w8],
                                    start=first, stop=last,
                                )

            gsb = perspool.tile([96, 192], dt.float32)
            if "pe" not in _ABLATE:
                for n in range(N):
                    nc.vector.tensor_copy(gsb[:, 96 * n : 96 * (n + 1)], g_ps[n][:])
            else:
                nc.vector.memset(gsb[:], 0.0)
            nc.sync.dma_start(gout[:], gsb[:])
            nc.sync.dma_start(lout[:], lacc[:])

    nc.finalize()
    return nc


def _prep_core(x, t, k):
    """Per-core device arrays. x: (N,C,H,W,Z) f32, t: (N,H,W,Z) int."""
    xc = np.ascontiguousarray(x[:, :, HSH * k : HSH * (k + 1)])      # (2,12,16,128,128)
    xd = (
        xc.reshape(N, C, 128, TPN, NW, MMW)
        .transpose(0, 3, 2, 4, 1, 5)                                  # n,t16,p,w8,c,w
        .reshape(NT, 128, 12 * WT)
        .astype(FP8)
    )
    tc_ = (
        t[:, HSH * k : HSH * (k + 1)]
        .reshape(N, 128, TPN, NW, MMW)
        .transpose(0, 2, 1, 3, 4)                                     # n,t16,p,w8,w
        .reshape(NT, 128, NW, MMW)
    )
    th = tc_[:, :, :, None, :] == np.arange(C, dtype=tc_.dtype)[None, None, None, :, None]
    thd = th.astype(FP8).reshape(NT, 128, 12 * WT)
    return xd, thd


def _decode(results):
    """Sum per-core G blocks -> Gp[n] (12x12) + logsum."""
    Gp = np.zeros((N, C, C), np.float64)
    logsum = 0.0
    for res in results:
        g = res["gout"].astype(np.float64)                            # [96, 192]
        for n in range(N):
            blk = g[:, 96 * n : 96 * (n + 1)].reshape(C, MMW, C, MMW)
            Gp[n] += np.einsum("awbw->ab", blk)
        logsum += float(res["lout"].astype(np.float64).sum())
    logsum += MITCH_B * NPIX    # accum carries only the bits*a part of ln s
    return Gp, logsum


def run(inputs, trace=False):
    x = np.asarray(inputs["input"], dtype=np.float32)
    t = np.asarray(inputs["target"])
    Mp = np.asarray(inputs["matrix_penalty"], dtype=np.float32)
    tt = np.asarray(t[:, 0])                                          # (N,H,W,Z) int

    if "nc" not in _CACHE:
        _CACHE["nc"] = _build_nc()
    nc = _CACHE["nc"]

    in_maps = []
    for k in range(NCORES):
        xd, thd = _prep_core(x, tt, k)
        in_maps.append({"xin": xd, "tin": thd})

    res = run_bass_kernel_spmd(nc, in_maps, core_ids=list(range(NCORES)), trace=trace)
    Gp, logsum = _decode(res.results)

    ground_o = np.stack(
        [np.bincount(tt[n].ravel().astype(np.int64), minlength=C) for n in range(N)]
    ).astype(np.float64)
    inter = np.einsum("ncc->nc", Gp)
    pred_o = Gp.sum(axis=1)
    xt_sum = float(
        np.take_along_axis(x, tt[:, None].astype(np.int64), axis=1).sum(dtype=np.float64)
    )

    ce = (logsum - xt_sum) / NPIX
    dice = np.mean(1.0 - (2.0 * inter + SMOOTH) / (ground_o + pred_o + SMOOTH))
    pen = BETA / NPIX * float((Mp[None] * Gp).sum())
    loss = np.float32(ce + dice + pen)
    return loss, res


def kernel(**inputs):
    return run(inputs)[0]
